# revision 1
# baseline (speedup 1.0000x reference)
"""DLRM forward on 8 Trainium2 NeuronCores (Bass/Tile).

Strategy (v2):
- z has 432 feature rows (16 dense + 26*16 sparse); 432 = 8*54. Core c owns
  z-rows [48c, 48c+48) (3 whole 16-row units: unit0=dense, unit u=table u-1)
  plus a 6-row piece [384+6c, 390+6c) of tables 23..25. Each core gathers its
  units (one indirect-DMA per 128-batch chunk; the piece may span two tables
  -> two gathers with host-zero-masked columns, accum-add), transposes to
  zT-shard [54, 512], then AllGather assembles the (permuted) zT on all cores.
- Interaction + pred layer 0: out[b,o] = sum_{i,j} z_i z_j pw0[(i,j),o],
  K-sharded by i over cores in shard order (host permutes pw0 row-blocks to
  match). Per i-block: broadcast-DMA one zT row from the core's OWN shard
  bounce (B_i), DVE-multiply with the zT j-tiles -> interT chunk, then 16
  float32r matmuls (N=512) accumulate out^T in 4 PSUM banks. pw0 streams
  fp32 via HWDGE (no SWDGE descriptor-generation cost).
- Partial out^T is AllReduced (bf16) and every core redundantly computes the
  prediction MLP tail + sigmoid; core 0's output is returned.
"""

import numpy as np
import ml_dtypes

BATCH = 512
CARD = 100000
ED = 16
NCORES = 8
S = 54           # z-rows per core
ZR = 432
O = 512

BF16 = ml_dtypes.bfloat16

_state = {}


def _build_module():
    import concourse.bass as bass
    import concourse.mybir as mybir
    import concourse.tile as tile
    from concourse import bacc
    from concourse.masks import make_identity

    dt = mybir.dt
    nc = bacc.Bacc("TRN2", target_bir_lowering=False, debug=False,
                   num_devices=NCORES)

    # host-packed pw0 slices (partition-major for big DMA descriptors):
    # pw0a: full main chunks, pw0b: 48-row tails, pw0ta/tb: transposed side
    # ---- per-core DRAM inputs ----
    pw0a = nc.dram_tensor("pw0a", [128, 96 * O], dt.float32, kind="ExternalInput").ap()
    pw0b = nc.dram_tensor("pw0b", [48, 54 * O], dt.float32, kind="ExternalInput").ap()
    pw0ta = nc.dram_tensor("pw0ta", [128, 48 * O], dt.float32, kind="ExternalInput").ap()
    pw0tb = nc.dram_tensor("pw0tb", [48, 48 * O], dt.float32, kind="ExternalInput").ap()
    embs = {}
    for nm in ("es0", "es1", "es2", "es3a", "es3b"):
        embs[nm] = nc.dram_tensor(nm, [CARD, ED], dt.bfloat16,
                                  kind="ExternalInput").ap()
    idxq = nc.dram_tensor("idxq", [128, 20], dt.int32, kind="ExternalInput").ap()
    dfT = nc.dram_tensor("dfT", [16, BATCH], dt.bfloat16, kind="ExternalInput").ap()
    dw0 = nc.dram_tensor("dw0", [16, 512], dt.bfloat16, kind="ExternalInput").ap()
    dw1 = nc.dram_tensor("dw1", [512, 256], dt.bfloat16, kind="ExternalInput").ap()
    dw2 = nc.dram_tensor("dw2", [256, 64], dt.bfloat16, kind="ExternalInput").ap()
    dw3 = nc.dram_tensor("dw3", [64, 16], dt.bfloat16, kind="ExternalInput").ap()
    db0q = nc.dram_tensor("db0q", [128, 4], dt.float32, kind="ExternalInput").ap()
    db1q = nc.dram_tensor("db1q", [128, 2], dt.float32, kind="ExternalInput").ap()
    db2q = nc.dram_tensor("db2q", [64, 1], dt.float32, kind="ExternalInput").ap()
    db3q = nc.dram_tensor("db3q", [16, 1], dt.float32, kind="ExternalInput").ap()
    pw1 = nc.dram_tensor("pw1", [512, 256], dt.bfloat16, kind="ExternalInput").ap()
    pw2 = nc.dram_tensor("pw2", [256, 1], dt.bfloat16, kind="ExternalInput").ap()
    pb0q = nc.dram_tensor("pb0q", [128, 4], dt.float32, kind="ExternalInput").ap()
    pb1q = nc.dram_tensor("pb1q", [128, 2], dt.float32, kind="ExternalInput").ap()
    pb2q = nc.dram_tensor("pb2q", [1, 1], dt.float32, kind="ExternalInput").ap()
    out_d = nc.dram_tensor("out", [1, BATCH], dt.float32, kind="ExternalOutput").ap()

    rg = [list(range(NCORES))]
    f32r = dt.float32r

    with tile.TileContext(nc) as tc:
        with tc.tile_pool(name="const", bufs=1) as cp, \
             tc.tile_pool(name="dram", bufs=1, space="DRAM") as dp:

            ag_in = dp.tile([S, BATCH], dt.bfloat16, tag="ag_in", name="ag_in")
            ag_out = dp.tile([ZR, BATCH], dt.bfloat16, tag="ag_out", name="ag_out")
            ar_in = dp.tile([O, BATCH], dt.bfloat16, tag="ar_in", name="ar_in")
            ar_out = dp.tile([O, BATCH], dt.bfloat16, tag="ar_out", name="ar_out")

            # ---- constants / small weights ----
            ident = cp.tile([128, 128], dt.bfloat16, tag="ident", name="ident")
            make_identity(nc, ident[:])
            idx_sb = cp.tile([128, 20], dt.int32, tag="idx_sb", name="idx_sb")
            nc.sync.dma_start(out=idx_sb[:], in_=idxq[:, :])
            dfT_sb = cp.tile([16, BATCH], dt.bfloat16, tag="dfT_sb", name="dfT_sb")
            nc.sync.dma_start(out=dfT_sb[:], in_=dfT[:, :])
            dw0_sb = cp.tile([16, 512], dt.bfloat16, tag="dw0_sb", name="dw0_sb")
            nc.sync.dma_start(out=dw0_sb[:], in_=dw0[:, :])
            dw1_sb = [cp.tile([128, 256], dt.bfloat16, tag=f"dw1_{k}", name=f"dw1_{k}")
                      for k in range(4)]
            for k in range(4):
                nc.sync.dma_start(out=dw1_sb[k][:], in_=dw1[k * 128:(k + 1) * 128, :])
            dw2_sb = [cp.tile([128, 64], dt.bfloat16, tag=f"dw2_{k}", name=f"dw2_{k}")
                      for k in range(2)]
            for k in range(2):
                nc.sync.dma_start(out=dw2_sb[k][:], in_=dw2[k * 128:(k + 1) * 128, :])
            dw3_sb = cp.tile([64, 16], dt.bfloat16, tag="dw3_sb", name="dw3_sb")
            nc.sync.dma_start(out=dw3_sb[:], in_=dw3[:, :])
            pw1_sb = [cp.tile([128, 256], dt.bfloat16, tag=f"pw1_{k}", name=f"pw1_{k}")
                      for k in range(4)]
            for k in range(4):
                nc.sync.dma_start(out=pw1_sb[k][:], in_=pw1[k * 128:(k + 1) * 128, :])
            pw2_sb = [cp.tile([128, 1], dt.bfloat16, tag=f"pw2_{k}", name=f"pw2_{k}")
                      for k in range(2)]
            for k in range(2):
                nc.sync.dma_start(out=pw2_sb[k][:], in_=pw2[k * 128:(k + 1) * 128, :])
            db0_sb = cp.tile([128, 4], dt.float32, tag="db0_sb", name="db0_sb")
            nc.sync.dma_start(out=db0_sb[:], in_=db0q[:, :])
            db1_sb = cp.tile([128, 2], dt.float32, tag="db1_sb", name="db1_sb")
            nc.sync.dma_start(out=db1_sb[:], in_=db1q[:, :])
            db2_sb = cp.tile([64, 1], dt.float32, tag="db2_sb", name="db2_sb")
            nc.sync.dma_start(out=db2_sb[:], in_=db2q[:, :])
            db3_sb = cp.tile([16, 1], dt.float32, tag="db3_sb", name="db3_sb")
            nc.sync.dma_start(out=db3_sb[:], in_=db3q[:, :])
            pb0_sb = cp.tile([128, 4], dt.float32, tag="pb0_sb", name="pb0_sb")
            nc.sync.dma_start(out=pb0_sb[:], in_=pb0q[:, :])
            pb1_sb = cp.tile([128, 2], dt.float32, tag="pb1_sb", name="pb1_sb")
            nc.sync.dma_start(out=pb1_sb[:], in_=pb1q[:, :])
            pb2_sb = cp.tile([1, 1], dt.float32, tag="pb2_sb", name="pb2_sb")
            nc.sync.dma_start(out=pb2_sb[:], in_=pb2q[:, :])

            # ---- gathers: slots 0-2 single table, slot 3 = two-half piece ----
            with tc.tile_pool(name="gather", bufs=1) as gp, \
                 tc.tile_pool(name="ps_g", bufs=1, space="PSUM") as pg, \
                 tc.tile_pool(name="ps_d", bufs=2, space="PSUM") as pd, \
                 tc.tile_pool(name="dmlp", bufs=1) as dm:
                zsl = []
                for s in range(4):
                    ps_s = pg.tile([16, BATCH], dt.bfloat16, tag=f"psg{s}",
                                   name=f"psg{s}")
                    for bc in range(4):
                        gt = gp.tile([128, ED], dt.bfloat16, tag=f"g{s}_{bc}",
                                     name=f"g{s}_{bc}")
                        if s < 3:
                            nc.gpsimd.indirect_dma_start(
                                out=gt[:], out_offset=None, in_=embs[f"es{s}"][:, :],
                                in_offset=bass.IndirectOffsetOnAxis(
                                    ap=idx_sb[:, s * 4 + bc:s * 4 + bc + 1], axis=0))
                        else:
                            nc.gpsimd.indirect_dma_start(
                                out=gt[:], out_offset=None, in_=embs["es3a"][:, :],
                                in_offset=bass.IndirectOffsetOnAxis(
                                    ap=idx_sb[:, 12 + bc:13 + bc], axis=0))
                            nc.gpsimd.indirect_dma_start(
                                out=gt[:], out_offset=None, in_=embs["es3b"][:, :],
                                in_offset=bass.IndirectOffsetOnAxis(
                                    ap=idx_sb[:, 16 + bc:17 + bc], axis=0),
                                compute_op=mybir.AluOpType.add)
                        nc.tensor.transpose(out=ps_s[:16, bc * 128:(bc + 1) * 128],
                                            in_=gt[:, :], identity=ident[:])
                    z_s = dm.tile([16, BATCH], dt.bfloat16, tag=f"zsl{s}",
                                  name=f"zsl{s}")
                    nc.vector.tensor_copy(out=z_s[:], in_=ps_s[:16, :])
                    zsl.append(z_s)

                # ---- dense MLP (exactly 0 on cores != 0 via zeroed dw3/db3) ----
                h1 = []
                for mc in range(4):
                    d1 = pd.tile([128, BATCH], dt.float32, tag="dscr", name="d1")
                    nc.tensor.matmul(out=d1[:], lhsT=dw0_sb[:, mc * 128:(mc + 1) * 128],
                                     rhs=dfT_sb[:], start=True, stop=True)
                    h = dm.tile([128, BATCH], dt.bfloat16, tag=f"h1_{mc}",
                                name=f"h1_{mc}")
                    nc.scalar.activation(out=h[:], in_=d1[:],
                                         func=mybir.ActivationFunctionType.Relu,
                                         bias=db0_sb[:, mc:mc + 1])
                    h1.append(h)
                h2 = []
                for mc in range(2):
                    d2 = pd.tile([128, BATCH], dt.float32, tag="dscr", name="d2")
                    for k in range(4):
                        nc.tensor.matmul(out=d2[:],
                                         lhsT=dw1_sb[k][:, mc * 128:(mc + 1) * 128],
                                         rhs=h1[k][:], start=(k == 0), stop=(k == 3))
                    h = dm.tile([128, BATCH], dt.bfloat16, tag=f"h2_{mc}",
                                name=f"h2_{mc}")
                    nc.scalar.activation(out=h[:], in_=d2[:],
                                         func=mybir.ActivationFunctionType.Relu,
                                         bias=db1_sb[:, mc:mc + 1])
                    h2.append(h)
                d3 = pd.tile([128, BATCH], dt.float32, tag="dscr", name="d3")
                for k in range(2):
                    nc.tensor.matmul(out=d3[:64, :], lhsT=dw2_sb[k][:, :],
                                     rhs=h2[k][:], start=(k == 0), stop=(k == 1))
                h3 = dm.tile([64, BATCH], dt.bfloat16, tag="h3", name="h3")
                nc.scalar.activation(out=h3[:], in_=d3[:64, :],
                                     func=mybir.ActivationFunctionType.Relu,
                                     bias=db2_sb[:, 0:1])
                d4 = pd.tile([128, BATCH], dt.float32, tag="dscr", name="d4")
                nc.tensor.matmul(out=d4[:16, :], lhsT=dw3_sb[:, :], rhs=h3[:],
                                 start=True, stop=True)
                dense_sb = dm.tile([16, BATCH], dt.bfloat16, tag="dense_sb",
                                   name="dense_sb")
                nc.vector.tensor_scalar_add(out=dense_sb[:], in0=d4[:16, :],
                                            scalar1=db3_sb[:, 0:1])
                nc.vector.tensor_add(out=zsl[0][:], in0=zsl[0][:], in1=dense_sb[:])

                for s in range(3):
                    nc.sync.dma_start(out=ag_in[16 * s:16 * s + 16, :], in_=zsl[s][:])
                nc.sync.dma_start(out=ag_in[48:54, :], in_=zsl[3][0:6, :])

            nc.gpsimd.collective_compute(
                "AllGather", mybir.AluOpType.bypass, replica_groups=rg,
                ins=[ag_in[:].opt()], outs=[ag_out[:].opt()])

            # zT j-tiles in true z-order from the permuted ag_out:
            # z-row j = 128q+16m+d (unit 8q+m owned by core m, slot q)
            #   -> ag row 54m + 16q + d;  j>=384: j=384+6c'+e -> 54c'+48+e
            zt = []
            for jc in range(3):
                t = cp.tile([128, BATCH], dt.bfloat16, tag=f"zt{jc}", name=f"zt{jc}")
                for mu in range(8):
                    nc.sync.dma_start(
                        out=t[16 * mu:16 * mu + 16, :],
                        in_=ag_out[54 * mu + 16 * jc:54 * mu + 16 * jc + 16, :])
                zt.append(t)
            t3 = cp.tile([48, BATCH], dt.bfloat16, tag="zt3", name="zt3")
            for cc in range(NCORES):
                nc.sync.dma_start(out=t3[6 * cc:6 * cc + 6, :],
                                  in_=ag_out[54 * cc + 48:54 * cc + 54, :])
            zt.append(t3)

            # ---- main loop: block-diagonal + symmetrized-upper chunks ----
            # il 0-15: q=0, 16-31: q=1, 32-47: q=2, 48-53: q=3
            with tc.tile_pool(name="wp", bufs=10) as wp, \
                 tc.tile_pool(name="tp", bufs=4) as tp_, \
                 tc.tile_pool(name="bp", bufs=4) as bp, \
                 tc.tile_pool(name="ip", bufs=6) as ip, \
                 tc.tile_pool(name="ps_acc", bufs=1, space="PSUM") as pa, \
                 tc.tile_pool(name="outp", bufs=1) as op_:

                acc = [pa.tile([128, BATCH], dt.float32, tag=f"acc{oc}",
                               name=f"acc{oc}") for oc in range(4)]

                pw0ar = pw0a.bitcast(f32r)
                pw0br = pw0b.bitcast(f32r)
                pw0tar = pw0ta.bitcast(f32r)
                pw0tbr = pw0tb.bitcast(f32r)
                coff = 0
                tcoff = 0
                for il in range(S):
                    q = min(il // 16, 3)
                    nch = 4 - q
                    nfull = 3 - q
                    wsl = wp.tile([128, 4 * O], f32r, tag="wsl", name="wsl")
                    if nfull > 0:
                        nc.sync.dma_start(out=wsl[:, 0:nfull * O],
                                          in_=pw0ar[:, coff:coff + nfull * O])
                        coff += nfull * O
                    nc.sync.dma_start(out=wsl[0:48, nfull * O:nch * O],
                                      in_=pw0br[:, il * O:(il + 1) * O])
                    if q < 3:
                        nsf = nfull - 1
                        stg = tp_.tile([128, 3 * O], f32r, tag="stg", name="stg")
                        if nsf > 0:
                            nc.sync.dma_start(out=stg[:, 0:nsf * O],
                                              in_=pw0tar[:, tcoff:tcoff + nsf * O])
                            tcoff += nsf * O
                        nc.sync.dma_start(out=stg[0:48, nsf * O:(nsf + 1) * O],
                                          in_=pw0tbr[:, il * O:(il + 1) * O])
                        for k in range(nsf):
                            eng = nc.gpsimd if (il + k) % 2 == 0 else nc.vector
                            eng.tensor_add(
                                out=wsl[:, (k + 1) * O:(k + 2) * O],
                                in0=wsl[:, (k + 1) * O:(k + 2) * O],
                                in1=stg[:, k * O:(k + 1) * O])
                        eng = nc.gpsimd if il % 2 == 0 else nc.vector
                        eng.tensor_add(
                            out=wsl[0:48, nfull * O:nch * O],
                            in0=wsl[0:48, nfull * O:nch * O],
                            in1=stg[0:48, nsf * O:(nsf + 1) * O])
                    b_t = bp.tile([128, BATCH], dt.bfloat16, tag="b_t", name="b_t")
                    nc.sync.dma_start(out=b_t[:],
                                      in_=ag_in[il:il + 1, :].to_broadcast([128, BATCH]))
                    for k in range(nch):
                        jcz = q + k
                        npart = 128 if jcz < 3 else 48
                        it = ip.tile([128, BATCH], f32r, tag="it", name="it")
                        nc.vector.tensor_mul(out=it[:npart, :], in0=zt[jcz][:npart, :],
                                             in1=b_t[:npart, :])
                        for oc in range(4):
                            lhsT = wsl[:npart, k * O + oc * 128:k * O + (oc + 1) * 128]
                            nc.tensor.matmul(
                                out=acc[oc][:], lhsT=lhsT,
                                rhs=it[:npart, :],
                                start=(il == 0 and k == 0),
                                stop=(il == S - 1 and k == 0))

                for oc in range(4):
                    osb = op_.tile([128, BATCH], dt.bfloat16, tag=f"osb{oc}",
                                   name=f"osb{oc}")
                    nc.scalar.activation(out=osb[:], in_=acc[oc][:],
                                         func=mybir.ActivationFunctionType.Copy)
                    nc.sync.dma_start(out=ar_in[oc * 128:(oc + 1) * 128, :], in_=osb[:])

            nc.gpsimd.collective_compute(
                "AllReduce", mybir.AluOpType.add, replica_groups=rg,
                ins=[ar_in[:].opt()], outs=[ar_out[:].opt()])

            # ---- prediction MLP tail ----
            with tc.tile_pool(name="tail_sb", bufs=1) as ts, \
                 tc.tile_pool(name="ps_t", bufs=1, space="PSUM") as pt:
                h0 = []
                for kc in range(4):
                    r = ts.tile([128, BATCH], dt.bfloat16, tag=f"red{kc}",
                                name=f"red{kc}")
                    nc.sync.dma_start(out=r[:], in_=ar_out[kc * 128:(kc + 1) * 128, :])
                    h = ts.tile([128, BATCH], dt.bfloat16, tag=f"h0_{kc}",
                                name=f"h0_{kc}")
                    nc.scalar.activation(out=h[:], in_=r[:],
                                         func=mybir.ActivationFunctionType.Relu,
                                         bias=pb0_sb[:, kc:kc + 1])
                    h0.append(h)
                h1p = []
                for mc in range(2):
                    p1 = pt.tile([128, BATCH], dt.float32, tag=f"p1_{mc}",
                                 name=f"p1_{mc}")
                    for kc in range(4):
                        nc.tensor.matmul(out=p1[:],
                                         lhsT=pw1_sb[kc][:, mc * 128:(mc + 1) * 128],
                                         rhs=h0[kc][:], start=(kc == 0), stop=(kc == 3))
                    h = ts.tile([128, BATCH], dt.bfloat16, tag=f"h1p_{mc}",
                                name=f"h1p_{mc}")
                    nc.scalar.activation(out=h[:], in_=p1[:],
                                         func=mybir.ActivationFunctionType.Relu,
                                         bias=pb1_sb[:, mc:mc + 1])
                    h1p.append(h)
                p2 = pt.tile([1, BATCH], dt.float32, tag="p2", name="p2")
                for mc in range(2):
                    nc.tensor.matmul(out=p2[:], lhsT=pw2_sb[mc][:, :], rhs=h1p[mc][:],
                                     start=(mc == 0), stop=(mc == 1))
                res = ts.tile([1, BATCH], dt.float32, tag="res", name="res")
                nc.scalar.activation(out=res[:], in_=p2[:],
                                     func=mybir.ActivationFunctionType.Sigmoid,
                                     bias=pb2_sb[:, 0:1])
                nc.sync.dma_start(out=out_d[:, :], in_=res[:])

    nc.compile()
    return nc


def _host_prep(inputs):
    f32 = np.float32
    df = np.asarray(inputs["dense_features"], f32)
    sf = np.asarray(inputs["sparse_features"])
    emb = np.asarray(inputs["emb"], f32)
    pw0 = np.asarray(inputs["pw0"], f32)

    idx = ((sf.astype(np.int64) + 1) % CARD).astype(np.int32)   # [512, 26]
    embb = emb.astype(BF16)                                     # [26, CARD, 16]
    pw0v = pw0.reshape(ZR, ZR, O)

    dfT = np.zeros((16, BATCH), BF16)
    dfT[:13] = df.T.astype(BF16)
    dw0p = np.zeros((16, 512), f32)
    dw0p[:13] = np.asarray(inputs["dw0"], f32)

    def col(b, p):
        return np.asarray(b, f32).reshape(p, 128).T.copy()

    common = {
        "dfT": dfT,
        "dw0": dw0p.astype(BF16),
        "dw1": np.asarray(inputs["dw1"], f32).astype(BF16),
        "dw2": np.asarray(inputs["dw2"], f32).astype(BF16),
        "db0q": col(inputs["db0"], 4),
        "db1q": col(inputs["db1"], 2),
        "db2q": np.asarray(inputs["db2"], f32).reshape(64, 1).copy(),
        "pw1": np.asarray(inputs["pw1"], f32).astype(BF16),
        "pw2": np.asarray(inputs["pw2"], f32).reshape(256, 1).astype(BF16),
        "pb0q": col(inputs["pb0"], 4),
        "pb1q": col(inputs["pb1"], 2),
        "pb2q": np.asarray(inputs["pb2"], f32).reshape(1, 1).copy(),
    }
    dw3 = np.asarray(inputs["dw3"], f32).astype(BF16)
    db3 = np.asarray(inputs["db3"], f32).reshape(16, 1).astype(f32)
    zero_tab = np.zeros((CARD, ED), BF16)
    zero_idx = np.zeros(BATCH, np.int32)

    in_maps = []
    for c in range(NCORES):
        m = dict(common)
        m["dw3"] = dw3 if c == 0 else np.zeros_like(dw3)
        m["db3q"] = db3 if c == 0 else np.zeros_like(db3)

        # shard z-rows: units {c, 8+c, 16+c} (16 rows each) + piece [384+6c, +6)
        def zrow(il):
            q = min(il // 16, 3)
            if q < 3:
                return 128 * q + 16 * c + (il - 16 * q)
            return 384 + 6 * c + (il - 48)

        # partition-major packing: per il, full main chunks as [128, nfull*512]
        # (row j=128q+128k+p -> [p, k*512:...]), 48-row tails row-major.
        pa_, pb_, pta, ptb = [], [], [], []
        for il in range(S):
            q = min(il // 16, 3)
            i = zrow(il)
            nfull = 3 - q
            if nfull > 0:
                blk = pw0v[i, 128 * q:128 * q + nfull * 128, :]
                pa_.append(blk.reshape(nfull, 128, O).transpose(1, 0, 2)
                           .reshape(128, nfull * O))
            pb_.append(pw0v[i, 384:432, :])
            if q < 3:
                nsf = nfull - 1
                if nsf > 0:
                    tb = pw0v[128 * (q + 1):128 * (q + 1) + nsf * 128, i, :]
                    pta.append(tb.reshape(nsf, 128, O).transpose(1, 0, 2)
                               .reshape(128, nsf * O))
                ptb.append(pw0v[384:432, i, :])
        m["pw0a"] = np.ascontiguousarray(np.concatenate(pa_, 1))
        m["pw0b"] = np.ascontiguousarray(np.concatenate(pb_, 1))
        m["pw0ta"] = np.ascontiguousarray(np.concatenate(pta, 1))
        m["pw0tb"] = np.ascontiguousarray(np.concatenate(ptb, 1))

        idx_cols = []
        for s in range(3):
            u = [c, 8 + c, 16 + c][s]   # unit; u==0 is dense
            if u == 0:
                m[f"es{s}"] = zero_tab
                idx_cols.append(zero_idx)
            else:
                m[f"es{s}"] = np.ascontiguousarray(embb[u - 1])
                idx_cols.append(idx[:, u - 1])
        # piece: cols e=0..5 <- table 23+(6c+e)//16, dim (6c+e)%16
        ta = 23 + (6 * c) // 16
        ea = np.zeros((CARD, ED), BF16)
        eb = np.zeros((CARD, ED), BF16)
        tb = None
        for e in range(6):
            t_ = 23 + (6 * c + e) // 16
            d_ = (6 * c + e) % 16
            if t_ == ta:
                ea[:, e] = embb[t_][:, d_]
            else:
                tb = t_
                eb[:, e] = embb[t_][:, d_]
        m["es3a"] = ea
        m["es3b"] = eb
        idx_cols.append(idx[:, ta])
        idx_cols.append(idx[:, tb] if tb is not None else zero_idx)

        iq = np.zeros((128, 20), np.int32)
        for sa in range(5):
            iq[:, sa * 4:(sa + 1) * 4] = idx_cols[sa].reshape(4, 128).T
        m["idxq"] = iq
        in_maps.append(m)
    return in_maps


def kernel(**inputs):
    from concourse import bass_utils
    import os

    if "nc" not in _state:
        _state["nc"] = _build_module()
    in_maps = _host_prep(inputs)
    trace = bool(int(os.environ.get("DLRM_TRACE", "0")))
    res = bass_utils.run_bass_kernel_spmd(
        _state["nc"], in_maps, core_ids=list(range(NCORES)), trace=trace)
    _state["last_results"] = res
    return np.asarray(res.results[0]["out"], np.float32).reshape(BATCH)



# revision 9
# speedup vs baseline: 1.1165x; 1.1165x over previous
"""DLRM forward on 8 Trainium2 NeuronCores (Bass/Tile).

Strategy (v3):
- Gather/dense/AllGather front-end as before: core c gathers its 3 whole
  tables (units c, 8+c, 16+c; unit 0 = dense arch output via zeroed-table
  trick) plus a 6-row piece of tables 24..26, AllGather assembles the
  (permuted) zT on all cores; zt tiles restore true z-row order.
- Interaction + pred layer 0 use host-FOLDED symmetric weights in bf16:
  out[o,b] = sum_{i<=j} Wf[(i,j),o] z_i[b] z_j[b], Wf = pw0[i,j]+pw0[j,i]
  (diag unfolded). Core c owns i-rows {128q+8d+c} u {384+8d+c} — the exact
  upper triangle, padded to a c-independent row count (pad rows get zero
  weight) so all 8 cores run one SPMD instruction stream. Rows are packed
  into 93 exact 128-row chunks; per chunk the interaction terms are built
  by DVE multiplies (zt j-slices x broadcast z_i) and contracted by 4
  N=512 matmuls into 4 PSUM banks. The full 11.9MB weight slab is
  prefetched to SBUF during the front-end.
- z_i rows are fetched data-driven (per-core row indices) via one indirect
  gather from ag_out -> SBUF -> DRAM bounce; b_i broadcast-DMAs read it.
- Partial out^T is AllReduced (bf16) and every core redundantly computes
  the prediction MLP tail + sigmoid; core 0's output is returned.
"""

import numpy as np
import ml_dtypes

BATCH = 512
CARD = 100000
ED = 16
NCORES = 8
NIL = 54         # interaction i-rows per core
ZR = 432
O = 512
RPC = 128        # interaction rows per chunk

BF16 = ml_dtypes.bfloat16

_state = {}


def _jstart(il):
    if il < 48:
        q, dd = divmod(il, 16)
        return 128 * q + 8 * dd
    return 384 + 8 * (il - 48)


def _i_of(il, c):
    return _jstart(il) + c


def _build_recipe():
    """Partition-aligned tile list, identical for all cores.

    Each tile is 128 interaction rows -> one it tile [128, 512] and 4
    matmuls. Every DVE multiply keeps out/in0/in1 on the same partitions:
    row at partition p always holds a j with p = j mod 128 (srcs zt0..zt2,
    zt3: p = j-384, t3b: p = j-384+64). ops: (p0, plen, src, il).
    jmap[p] = true j of that row (-1 = zero-weight gap), ilmap[p] = il.
    """
    tiles = []       # list of op-lists
    jmaps = []       # [128] per tile
    ilmaps = []
    leftover = []    # (il, jstart, len) tail pieces packed at the end
    for q in range(3):
        for d in range(16):
            il = 16 * q + d
            # diagonal tile, op start rounded down to a 32-boundary
            # (extra low rows have j < i -> zero weight), plus the il's
            # 48-row tail in the K-slack when it fits
            a = 32 * (d // 4)
            ops = [(a, 128 - a, f"zt{q}", il)]
            jm = [-1] * 128
            im = [il] * 128
            for p in range(a, 128):
                jm[p] = 128 * q + p
            if a >= 48:
                ops.append((0, 48, "zt3", il))
                for p in range(48):
                    jm[p] = 384 + p
            else:
                leftover.append((il, 384, 48))
            tiles.append(ops); jmaps.append(jm); ilmaps.append(im)
            # full j-block tiles above the diagonal block
            for t in range(q + 1, 3):
                tiles.append([(0, 128, f"zt{t}", il)])
                jmaps.append([128 * t + p for p in range(128)])
                ilmaps.append([il] * 128)
    for il in range(48, NIL):
        leftover.append((il, 384 + 8 * (il - 48), 48 - 8 * (il - 48)))
    # leftover: two windows per tile: [0,48) via zt3 (j=384+p),
    # [64,112) via t3b (j=320+p); starts rounded down to 32-boundaries
    for a in range(0, len(leftover), 2):
        ops = []
        jm = [-1] * 128
        im = [leftover[a][0]] * 128
        il0, js0, ln0 = leftover[a]
        pe = js0 - 384
        p0 = 32 * (pe // 32)
        ops.append((p0, ln0 + pe - p0, "zt3", il0))
        for p in range(p0, pe + ln0):
            jm[p] = 384 + p
            im[p] = il0
        if a + 1 < len(leftover):
            il1, js1, ln1 = leftover[a + 1]
            pe = js1 - 384 + 64
            p0 = 64 + 32 * ((js1 - 384) // 32)
            ops.append((p0, ln1 + pe - p0, "t3b", il1))
            for p in range(p0, pe + ln1):
                jm[p] = 320 + p
                im[p] = il1
        tiles.append(ops); jmaps.append(jm); ilmaps.append(im)

    # split ops so each stays in an aligned partition block:
    # base 0 -> <=128, base 64 -> <=64, base 32/96 -> <=32
    def lim(b):
        return 128 if b == 0 else (64 if b == 64 else 32)
    tiles2 = []
    for ops in tiles:
        o2 = []
        for (p0, plen, src, il) in ops:
            while plen > 0:
                take = min(plen, {0: 128, 32: 32, 64: 64, 96: 32}[p0])
                o2.append((p0, take, src, il))
                p0 += take
                plen -= take
        tiles2.append(o2)
    return tiles2, jmaps, ilmaps


TILES, JMAPS, ILMAPS = _build_recipe()
NCH = len(TILES)


def _perm(i):
    """Position of true z-row i inside ag_out."""
    if i < 384:
        u = i // 16
        return 54 * (u % 8) + 16 * (u // 8) + (i % 16)
    e = i - 384
    return 54 * (e // 6) + 48 + (e % 6)


def _build_module():
    import concourse.bass as bass
    import concourse.mybir as mybir
    import concourse.tile as tile
    from concourse import bacc
    from concourse.masks import make_identity

    dt = mybir.dt
    nc = bacc.Bacc("TRN2", target_bir_lowering=False, debug=False,
                   num_devices=NCORES)

    # ---- per-core DRAM inputs ----
    wsb_d = nc.dram_tensor("wsb", [128, NCH * O], dt.bfloat16,
                           kind="ExternalInput").ap()
    zidx = nc.dram_tensor("zidx", [NIL, 1], dt.int32, kind="ExternalInput").ap()
    embs = {}
    for nm in ("es0", "es1", "es2", "es3a", "es3b"):
        embs[nm] = nc.dram_tensor(nm, [CARD, ED], dt.bfloat16,
                                  kind="ExternalInput").ap()
    idxq = nc.dram_tensor("idxq", [128, 20], dt.int32, kind="ExternalInput").ap()
    dfT = nc.dram_tensor("dfT", [16, BATCH], dt.bfloat16, kind="ExternalInput").ap()
    dw0 = nc.dram_tensor("dw0", [16, 512], dt.bfloat16, kind="ExternalInput").ap()
    dw1 = nc.dram_tensor("dw1", [512, 256], dt.bfloat16, kind="ExternalInput").ap()
    dw2 = nc.dram_tensor("dw2", [256, 64], dt.bfloat16, kind="ExternalInput").ap()
    dw3 = nc.dram_tensor("dw3", [64, 16], dt.bfloat16, kind="ExternalInput").ap()
    db0q = nc.dram_tensor("db0q", [128, 4], dt.float32, kind="ExternalInput").ap()
    db1q = nc.dram_tensor("db1q", [128, 2], dt.float32, kind="ExternalInput").ap()
    db2q = nc.dram_tensor("db2q", [64, 1], dt.float32, kind="ExternalInput").ap()
    db3q = nc.dram_tensor("db3q", [16, 1], dt.float32, kind="ExternalInput").ap()
    pw1 = nc.dram_tensor("pw1", [512, 256], dt.bfloat16, kind="ExternalInput").ap()
    pw2 = nc.dram_tensor("pw2", [256, 1], dt.bfloat16, kind="ExternalInput").ap()
    pb0q = nc.dram_tensor("pb0q", [128, 4], dt.float32, kind="ExternalInput").ap()
    pb1q = nc.dram_tensor("pb1q", [128, 2], dt.float32, kind="ExternalInput").ap()
    pb2q = nc.dram_tensor("pb2q", [1, 1], dt.float32, kind="ExternalInput").ap()
    out_d = nc.dram_tensor("out", [1, BATCH], dt.float32, kind="ExternalOutput").ap()

    rg = [list(range(NCORES))]

    with tile.TileContext(nc) as tc:
        with tc.tile_pool(name="const", bufs=1) as cp, \
             tc.tile_pool(name="dram", bufs=1, space="DRAM") as dp:

            ag_in = dp.tile([NIL, BATCH], dt.bfloat16, tag="ag_in", name="ag_in")
            ag_out = dp.tile([ZR, BATCH], dt.bfloat16, tag="ag_out", name="ag_out")
            zi_d = dp.tile([NIL, BATCH], dt.bfloat16, tag="zi_d", name="zi_d")
            ar_in = dp.tile([O, BATCH], dt.bfloat16, tag="ar_in", name="ar_in")
            ar_out = dp.tile([O, BATCH], dt.bfloat16, tag="ar_out", name="ar_out")

            # ---- weight slab prefetch (overlaps the whole front-end) ----
            wsb = cp.tile([128, NCH * O], dt.bfloat16, tag="wsb", name="wsb")
            PFC = 8  # chunks per prefetch DMA
            for t0 in range(0, NCH, PFC):
                t1 = min(NCH, t0 + PFC)
                nc.sync.dma_start(out=wsb[:, t0 * O:t1 * O],
                                  in_=wsb_d[:, t0 * O:t1 * O])

            # ---- constants / small weights ----
            ident = cp.tile([128, 128], dt.bfloat16, tag="ident", name="ident")
            make_identity(nc, ident[:])
            idx_sb = cp.tile([128, 20], dt.int32, tag="idx_sb", name="idx_sb")
            nc.sync.dma_start(out=idx_sb[:], in_=idxq[:, :])
            zidx_sb = cp.tile([NIL, 1], dt.int32, tag="zidx_sb", name="zidx_sb")
            nc.sync.dma_start(out=zidx_sb[:], in_=zidx[:, :])
            dfT_sb = cp.tile([16, BATCH], dt.bfloat16, tag="dfT_sb", name="dfT_sb")
            nc.sync.dma_start(out=dfT_sb[:], in_=dfT[:, :])
            dw0_sb = cp.tile([16, 512], dt.bfloat16, tag="dw0_sb", name="dw0_sb")
            nc.sync.dma_start(out=dw0_sb[:], in_=dw0[:, :])
            dw1_sb = [cp.tile([128, 256], dt.bfloat16, tag=f"dw1_{k}", name=f"dw1_{k}")
                      for k in range(4)]
            for k in range(4):
                nc.sync.dma_start(out=dw1_sb[k][:], in_=dw1[k * 128:(k + 1) * 128, :])
            dw2_sb = [cp.tile([128, 64], dt.bfloat16, tag=f"dw2_{k}", name=f"dw2_{k}")
                      for k in range(2)]
            for k in range(2):
                nc.sync.dma_start(out=dw2_sb[k][:], in_=dw2[k * 128:(k + 1) * 128, :])
            dw3_sb = cp.tile([64, 16], dt.bfloat16, tag="dw3_sb", name="dw3_sb")
            nc.sync.dma_start(out=dw3_sb[:], in_=dw3[:, :])
            pw1_sb = [cp.tile([128, 256], dt.bfloat16, tag=f"pw1_{k}", name=f"pw1_{k}")
                      for k in range(4)]
            for k in range(4):
                nc.sync.dma_start(out=pw1_sb[k][:], in_=pw1[k * 128:(k + 1) * 128, :])
            pw2_sb = [cp.tile([128, 1], dt.bfloat16, tag=f"pw2_{k}", name=f"pw2_{k}")
                      for k in range(2)]
            for k in range(2):
                nc.sync.dma_start(out=pw2_sb[k][:], in_=pw2[k * 128:(k + 1) * 128, :])
            db0_sb = cp.tile([128, 4], dt.float32, tag="db0_sb", name="db0_sb")
            nc.sync.dma_start(out=db0_sb[:], in_=db0q[:, :])
            db1_sb = cp.tile([128, 2], dt.float32, tag="db1_sb", name="db1_sb")
            nc.sync.dma_start(out=db1_sb[:], in_=db1q[:, :])
            db2_sb = cp.tile([64, 1], dt.float32, tag="db2_sb", name="db2_sb")
            nc.sync.dma_start(out=db2_sb[:], in_=db2q[:, :])
            db3_sb = cp.tile([16, 1], dt.float32, tag="db3_sb", name="db3_sb")
            nc.sync.dma_start(out=db3_sb[:], in_=db3q[:, :])
            pb0_sb = cp.tile([128, 4], dt.float32, tag="pb0_sb", name="pb0_sb")
            nc.sync.dma_start(out=pb0_sb[:], in_=pb0q[:, :])
            pb1_sb = cp.tile([128, 2], dt.float32, tag="pb1_sb", name="pb1_sb")
            nc.sync.dma_start(out=pb1_sb[:], in_=pb1q[:, :])
            pb2_sb = cp.tile([1, 1], dt.float32, tag="pb2_sb", name="pb2_sb")
            nc.sync.dma_start(out=pb2_sb[:], in_=pb2q[:, :])

            # ---- gathers: slots 0-2 single table, slot 3 = two-half piece ----
            with tc.tile_pool(name="gather", bufs=1) as gp, \
                 tc.tile_pool(name="ps_g", bufs=1, space="PSUM") as pg, \
                 tc.tile_pool(name="ps_d", bufs=2, space="PSUM") as pd, \
                 tc.tile_pool(name="dmlp", bufs=1) as dm:
                zsl = []
                for s in range(4):
                    ps_s = pg.tile([16, BATCH], dt.bfloat16, tag=f"psg{s}",
                                   name=f"psg{s}")
                    for bc in range(4):
                        gt = gp.tile([128, ED], dt.bfloat16, tag=f"g{s}_{bc}",
                                     name=f"g{s}_{bc}")
                        if s < 3:
                            nc.gpsimd.indirect_dma_start(
                                out=gt[:], out_offset=None, in_=embs[f"es{s}"][:, :],
                                in_offset=bass.IndirectOffsetOnAxis(
                                    ap=idx_sb[:, s * 4 + bc:s * 4 + bc + 1], axis=0))
                        else:
                            nc.gpsimd.indirect_dma_start(
                                out=gt[:], out_offset=None, in_=embs["es3a"][:, :],
                                in_offset=bass.IndirectOffsetOnAxis(
                                    ap=idx_sb[:, 12 + bc:13 + bc], axis=0))
                            nc.gpsimd.indirect_dma_start(
                                out=gt[:], out_offset=None, in_=embs["es3b"][:, :],
                                in_offset=bass.IndirectOffsetOnAxis(
                                    ap=idx_sb[:, 16 + bc:17 + bc], axis=0),
                                compute_op=mybir.AluOpType.add)
                        nc.tensor.transpose(out=ps_s[:16, bc * 128:(bc + 1) * 128],
                                            in_=gt[:, :], identity=ident[:])
                    z_s = dm.tile([16, BATCH], dt.bfloat16, tag=f"zsl{s}",
                                  name=f"zsl{s}")
                    nc.vector.tensor_copy(out=z_s[:], in_=ps_s[:16, :])
                    zsl.append(z_s)

                # ---- dense MLP (exactly 0 on cores != 0 via zeroed dw3/db3) ----
                h1 = []
                for mc in range(4):
                    d1 = pd.tile([128, BATCH], dt.float32, tag="dscr", name="d1")
                    nc.tensor.matmul(out=d1[:], lhsT=dw0_sb[:, mc * 128:(mc + 1) * 128],
                                     rhs=dfT_sb[:], start=True, stop=True)
                    h = dm.tile([128, BATCH], dt.bfloat16, tag=f"h1_{mc}",
                                name=f"h1_{mc}")
                    nc.scalar.activation(out=h[:], in_=d1[:],
                                         func=mybir.ActivationFunctionType.Relu,
                                         bias=db0_sb[:, mc:mc + 1])
                    h1.append(h)
                h2 = []
                for mc in range(2):
                    d2 = pd.tile([128, BATCH], dt.float32, tag="dscr", name="d2")
                    for k in range(4):
                        nc.tensor.matmul(out=d2[:],
                                         lhsT=dw1_sb[k][:, mc * 128:(mc + 1) * 128],
                                         rhs=h1[k][:], start=(k == 0), stop=(k == 3))
                    h = dm.tile([128, BATCH], dt.bfloat16, tag=f"h2_{mc}",
                                name=f"h2_{mc}")
                    nc.scalar.activation(out=h[:], in_=d2[:],
                                         func=mybir.ActivationFunctionType.Relu,
                                         bias=db1_sb[:, mc:mc + 1])
                    h2.append(h)
                d3 = pd.tile([128, BATCH], dt.float32, tag="dscr", name="d3")
                for k in range(2):
                    nc.tensor.matmul(out=d3[:64, :], lhsT=dw2_sb[k][:, :],
                                     rhs=h2[k][:], start=(k == 0), stop=(k == 1))
                h3 = dm.tile([64, BATCH], dt.bfloat16, tag="h3", name="h3")
                nc.scalar.activation(out=h3[:], in_=d3[:64, :],
                                     func=mybir.ActivationFunctionType.Relu,
                                     bias=db2_sb[:, 0:1])
                d4 = pd.tile([128, BATCH], dt.float32, tag="dscr", name="d4")
                nc.tensor.matmul(out=d4[:16, :], lhsT=dw3_sb[:, :], rhs=h3[:],
                                 start=True, stop=True)
                dense_sb = dm.tile([16, BATCH], dt.bfloat16, tag="dense_sb",
                                   name="dense_sb")
                nc.vector.tensor_scalar_add(out=dense_sb[:], in0=d4[:16, :],
                                            scalar1=db3_sb[:, 0:1])
                nc.vector.tensor_add(out=zsl[0][:], in0=zsl[0][:], in1=dense_sb[:])

                for s in range(3):
                    nc.sync.dma_start(out=ag_in[16 * s:16 * s + 16, :], in_=zsl[s][:])
                nc.sync.dma_start(out=ag_in[48:54, :], in_=zsl[3][0:6, :])

            nc.gpsimd.collective_compute(
                "AllGather", mybir.AluOpType.bypass, replica_groups=rg,
                ins=[ag_in[:].opt()], outs=[ag_out[:].opt()])

            # zT j-tiles in true z-order from the permuted ag_out:
            # z-row j = 128q+16m+d (unit 8q+m owned by core m, slot q)
            #   -> ag row 54m + 16q + d;  j>=384: j=384+6c'+e -> 54c'+48+e
            zt = []
            for jc in range(3):
                t = cp.tile([128, BATCH], dt.bfloat16, tag=f"zt{jc}", name=f"zt{jc}")
                for mu in range(8):
                    nc.sync.dma_start(
                        out=t[16 * mu:16 * mu + 16, :],
                        in_=ag_out[54 * mu + 16 * jc:54 * mu + 16 * jc + 16, :])
                zt.append(t)
            t3 = cp.tile([48, BATCH], dt.bfloat16, tag="zt3", name="zt3")
            for cc in range(NCORES):
                nc.sync.dma_start(out=t3[6 * cc:6 * cc + 6, :],
                                  in_=ag_out[54 * cc + 48:54 * cc + 54, :])
            zt.append(t3)
            t3b = cp.tile([112, BATCH], dt.bfloat16, tag="t3b", name="t3b")
            for cc in range(NCORES):
                nc.sync.dma_start(out=t3b[64 + 6 * cc:70 + 6 * cc, :],
                                  in_=ag_out[54 * cc + 48:54 * cc + 54, :])

            # per-core z_i rows -> DRAM bounce for broadcast reads
            ziT = cp.tile([NIL, BATCH], dt.bfloat16, tag="ziT", name="ziT")
            nc.gpsimd.indirect_dma_start(
                out=ziT[:], out_offset=None, in_=ag_out[:, :],
                in_offset=bass.IndirectOffsetOnAxis(ap=zidx_sb[:, 0:1], axis=0))
            nc.sync.dma_start(out=zi_d[:, :], in_=ziT[:])

            # ---- main loop: exact-triangle chunks ----
            with tc.tile_pool(name="bp", bufs=6) as bp, \
                 tc.tile_pool(name="ip", bufs=4) as ip, \
                 tc.tile_pool(name="ps_acc", bufs=1, space="PSUM") as pa, \
                 tc.tile_pool(name="outp", bufs=1) as op_:

                acc = [pa.tile([128, BATCH], dt.float32, tag=f"acc{oc}",
                               name=f"acc{oc}") for oc in range(4)]

                srcs = {"zt0": zt[0], "zt1": zt[1], "zt2": zt[2],
                        "zt3": zt[3], "t3b": t3b}
                # zero the it ring once so gap partitions are finite
                its = [ip.tile([128, BATCH], dt.bfloat16, tag="it", name="it")
                       for _ in range(4)]
                for t in its:
                    nc.vector.memset(t[:], 0)

                cur_il = -1
                b = None
                segctr = 0
                for k in range(NCH):
                    it = its[k % 4] if k < 4 else ip.tile(
                        [128, BATCH], dt.bfloat16, tag="it", name="it")
                    for (p0, plen, src, il) in TILES[k]:
                        if il != cur_il:
                            b = bp.tile([128, BATCH], dt.bfloat16, tag="b_t",
                                        name="b_t")
                            nc.sync.dma_start(
                                out=b[:],
                                in_=zi_d[il:il + 1, :].to_broadcast([128, BATCH]))
                            cur_il = il
                        eng = nc.gpsimd if segctr % 3 == 2 else nc.vector
                        eng.tensor_mul(out=it[p0:p0 + plen, :],
                                       in0=srcs[src][p0:p0 + plen, :],
                                       in1=b[p0:p0 + plen, :])
                        segctr += 1
                    for oc in range(4):
                        nc.tensor.matmul(
                            out=acc[oc][:],
                            lhsT=wsb[:, k * O + oc * 128:k * O + (oc + 1) * 128],
                            rhs=it[:, :],
                            start=(k == 0), stop=(k == NCH - 1))

                for oc in range(4):
                    osb = op_.tile([128, BATCH], dt.bfloat16, tag=f"osb{oc}",
                                   name=f"osb{oc}")
                    nc.scalar.activation(out=osb[:], in_=acc[oc][:],
                                         func=mybir.ActivationFunctionType.Copy)
                    nc.sync.dma_start(out=ar_in[oc * 128:(oc + 1) * 128, :], in_=osb[:])

            nc.gpsimd.collective_compute(
                "AllReduce", mybir.AluOpType.add, replica_groups=rg,
                ins=[ar_in[:].opt()], outs=[ar_out[:].opt()])

            # ---- prediction MLP tail ----
            with tc.tile_pool(name="tail_sb", bufs=1) as ts, \
                 tc.tile_pool(name="ps_t", bufs=1, space="PSUM") as pt:
                h0 = []
                for kc in range(4):
                    r = ts.tile([128, BATCH], dt.bfloat16, tag=f"red{kc}",
                                name=f"red{kc}")
                    nc.sync.dma_start(out=r[:], in_=ar_out[kc * 128:(kc + 1) * 128, :])
                    h = ts.tile([128, BATCH], dt.bfloat16, tag=f"h0_{kc}",
                                name=f"h0_{kc}")
                    nc.scalar.activation(out=h[:], in_=r[:],
                                         func=mybir.ActivationFunctionType.Relu,
                                         bias=pb0_sb[:, kc:kc + 1])
                    h0.append(h)
                h1p = []
                for mc in range(2):
                    p1 = pt.tile([128, BATCH], dt.float32, tag=f"p1_{mc}",
                                 name=f"p1_{mc}")
                    for kc in range(4):
                        nc.tensor.matmul(out=p1[:],
                                         lhsT=pw1_sb[kc][:, mc * 128:(mc + 1) * 128],
                                         rhs=h0[kc][:], start=(kc == 0), stop=(kc == 3))
                    h = ts.tile([128, BATCH], dt.bfloat16, tag=f"h1p_{mc}",
                                name=f"h1p_{mc}")
                    nc.scalar.activation(out=h[:], in_=p1[:],
                                         func=mybir.ActivationFunctionType.Relu,
                                         bias=pb1_sb[:, mc:mc + 1])
                    h1p.append(h)
                p2 = pt.tile([1, BATCH], dt.float32, tag="p2", name="p2")
                for mc in range(2):
                    nc.tensor.matmul(out=p2[:], lhsT=pw2_sb[mc][:, :], rhs=h1p[mc][:],
                                     start=(mc == 0), stop=(mc == 1))
                res = ts.tile([1, BATCH], dt.float32, tag="res", name="res")
                nc.scalar.activation(out=res[:], in_=p2[:],
                                     func=mybir.ActivationFunctionType.Sigmoid,
                                     bias=pb2_sb[:, 0:1])
                nc.sync.dma_start(out=out_d[:, :], in_=res[:])

    nc.compile()
    return nc


def _host_prep(inputs):
    f32 = np.float32
    df = np.asarray(inputs["dense_features"], f32)
    sf = np.asarray(inputs["sparse_features"])
    emb = np.asarray(inputs["emb"], f32)
    pw0 = np.asarray(inputs["pw0"], f32)

    idx = ((sf.astype(np.int64) + 1) % CARD).astype(np.int32)   # [512, 26]
    embb = emb.astype(BF16)                                     # [26, CARD, 16]

    # folded symmetric interaction weights (diag unfolded), bf16
    pw0v = pw0.reshape(ZR, ZR, O)
    Wfull = pw0v + pw0v.transpose(1, 0, 2)
    ar = np.arange(ZR)
    Wfull[ar, ar] = pw0v[ar, ar]
    Wb = Wfull.astype(BF16)                                     # [432, 432, 512]
    del Wfull

    dfT = np.zeros((16, BATCH), BF16)
    dfT[:13] = df.T.astype(BF16)
    dw0p = np.zeros((16, 512), f32)
    dw0p[:13] = np.asarray(inputs["dw0"], f32)

    def col(b, p):
        return np.asarray(b, f32).reshape(p, 128).T.copy()

    common = {
        "dfT": dfT,
        "dw0": dw0p.astype(BF16),
        "dw1": np.asarray(inputs["dw1"], f32).astype(BF16),
        "dw2": np.asarray(inputs["dw2"], f32).astype(BF16),
        "db0q": col(inputs["db0"], 4),
        "db1q": col(inputs["db1"], 2),
        "db2q": np.asarray(inputs["db2"], f32).reshape(64, 1).copy(),
        "pw1": np.asarray(inputs["pw1"], f32).astype(BF16),
        "pw2": np.asarray(inputs["pw2"], f32).reshape(256, 1).astype(BF16),
        "pb0q": col(inputs["pb0"], 4),
        "pb1q": col(inputs["pb1"], 2),
        "pb2q": np.asarray(inputs["pb2"], f32).reshape(1, 1).copy(),
    }
    dw3 = np.asarray(inputs["dw3"], f32).astype(BF16)
    db3 = np.asarray(inputs["db3"], f32).reshape(16, 1).astype(f32)
    zero_tab = np.zeros((CARD, ED), BF16)
    zero_idx = np.zeros(BATCH, np.int32)

    # c-independent row template from the tile maps
    Jt = np.asarray(JMAPS, np.int64).reshape(-1)       # [NCH*128], -1 = gap
    ILt = np.asarray(ILMAPS, np.int64).reshape(-1)
    JSt = np.array([_jstart(il) for il in range(NIL)], np.int64)[ILt]

    in_maps = []
    for c in range(NCORES):
        m = dict(common)
        m["dw3"] = dw3 if c == 0 else np.zeros_like(dw3)
        m["db3q"] = db3 if c == 0 else np.zeros_like(db3)

        I = JSt + c                          # i = jstart + c
        Wc = Wb[I, np.maximum(Jt, 0)]        # [NCH*128, 512] bf16
        Wc[(Jt < 0) | (Jt < I)] = 0
        m["wsb"] = np.ascontiguousarray(
            Wc.reshape(NCH, RPC, O).transpose(1, 0, 2).reshape(128, NCH * O))

        m["zidx"] = np.array([[_perm(_i_of(il, c))] for il in range(NIL)],
                             np.int32)

        idx_cols = []
        for s in range(3):
            u = [c, 8 + c, 16 + c][s]   # unit; u==0 is dense
            if u == 0:
                m[f"es{s}"] = zero_tab
                idx_cols.append(zero_idx)
            else:
                m[f"es{s}"] = np.ascontiguousarray(embb[u - 1])
                idx_cols.append(idx[:, u - 1])
        # piece: cols e=0..5 <- table 23+(6c+e)//16, dim (6c+e)%16
        ta = 23 + (6 * c) // 16
        ea = np.zeros((CARD, ED), BF16)
        eb = np.zeros((CARD, ED), BF16)
        tb = None
        for e in range(6):
            t_ = 23 + (6 * c + e) // 16
            d_ = (6 * c + e) % 16
            if t_ == ta:
                ea[:, e] = embb[t_][:, d_]
            else:
                tb = t_
                eb[:, e] = embb[t_][:, d_]
        m["es3a"] = ea
        m["es3b"] = eb
        idx_cols.append(idx[:, ta])
        idx_cols.append(idx[:, tb] if tb is not None else zero_idx)

        iq = np.zeros((128, 20), np.int32)
        for sa in range(5):
            iq[:, sa * 4:(sa + 1) * 4] = idx_cols[sa].reshape(4, 128).T
        m["idxq"] = iq
        in_maps.append(m)
    return in_maps


def kernel(**inputs):
    from concourse import bass_utils
    import os

    if "nc" not in _state:
        _state["nc"] = _build_module()
    in_maps = _host_prep(inputs)
    trace = bool(int(os.environ.get("DLRM_TRACE", "0")))
    res = bass_utils.run_bass_kernel_spmd(
        _state["nc"], in_maps, core_ids=list(range(NCORES)), trace=trace)
    _state["last_results"] = res
    return np.asarray(res.results[0]["out"], np.float32).reshape(BATCH)


# revision 17
# speedup vs baseline: 1.3471x; 1.2065x over previous
"""DLRM forward on 8 Trainium2 NeuronCores (Bass/Tile).

Strategy (v3):
- Gather/dense/AllGather front-end as before: core c gathers its 3 whole
  tables (units c, 8+c, 16+c; unit 0 = dense arch output via zeroed-table
  trick) plus a 6-row piece of tables 24..26, AllGather assembles the
  (permuted) zT on all cores; zt tiles restore true z-row order.
- Interaction + pred layer 0 use host-FOLDED symmetric weights in bf16:
  out[o,b] = sum_{i<=j} Wf[(i,j),o] z_i[b] z_j[b], Wf = pw0[i,j]+pw0[j,i]
  (diag unfolded). Core c owns i-rows {128q+8d+c} u {384+8d+c} — the exact
  upper triangle, padded to a c-independent row count (pad rows get zero
  weight) so all 8 cores run one SPMD instruction stream. Rows are packed
  into 93 exact 128-row chunks; per chunk the interaction terms are built
  by DVE multiplies (zt j-slices x broadcast z_i) and contracted by 4
  N=512 matmuls into 4 PSUM banks. The full 11.9MB weight slab is
  prefetched to SBUF during the front-end.
- z_i rows are fetched data-driven (per-core row indices) via one indirect
  gather from ag_out -> SBUF -> DRAM bounce; b_i broadcast-DMAs read it.
- Partial out^T is AllReduced (bf16) and every core redundantly computes
  the prediction MLP tail + sigmoid; core 0's output is returned.
"""

import numpy as np
import ml_dtypes

BATCH = 512
CARD = 100000
ED = 16
NCORES = 8
NIL = 54         # interaction i-rows per core
ZR = 432
O = 512
RPC = 128        # interaction rows per chunk

BF16 = ml_dtypes.bfloat16

_state = {}


def _jstart(il):
    if il < 48:
        q, dd = divmod(il, 16)
        return 128 * q + 8 * dd
    return 384 + 8 * (il - 48)


def _i_of(il, c):
    return _jstart(il) + c


def _build_recipe():
    """Partition-aligned tile list, identical for all cores.

    Each tile is 128 interaction rows -> one it tile [128, 512] and 4
    matmuls. Every DVE multiply keeps out/in0/in1 on the same partitions:
    row at partition p always holds a j with p = j mod 128 (srcs zt0..zt2,
    zt3: p = j-384, t3b: p = j-384+64). ops: (p0, plen, src, il).
    jmap[p] = true j of that row (-1 = zero-weight gap), ilmap[p] = il.
    """
    tiles = []       # list of op-lists
    jmaps = []       # [128] per tile
    ilmaps = []
    leftover = []    # (il, jstart, len) tail pieces packed at the end
    for q in range(3):
        for d in range(16):
            il = 16 * q + d
            # diagonal tile, op start rounded down to a 32-boundary
            # (extra low rows have j < i -> zero weight), plus the il's
            # 48-row tail in the K-slack when it fits
            a = 32 * (d // 4)
            ops = [(a, 128 - a, f"zt{q}", il)]
            jm = [-1] * 128
            im = [il] * 128
            for p in range(a, 128):
                jm[p] = 128 * q + p
            if a >= 48:
                ops.append((0, 48, "zt3", il))
                for p in range(48):
                    jm[p] = 384 + p
            else:
                leftover.append((il, 384, 48))
            tiles.append(ops); jmaps.append(jm); ilmaps.append(im)
            # full j-block tiles above the diagonal block
            for t in range(q + 1, 3):
                tiles.append([(0, 128, f"zt{t}", il)])
                jmaps.append([128 * t + p for p in range(128)])
                ilmaps.append([il] * 128)
    for il in range(48, NIL):
        leftover.append((il, 384 + 8 * (il - 48), 48 - 8 * (il - 48)))
    # leftover: two windows per tile: [0,48) via zt3 (j=384+p),
    # [64,112) via t3b (j=320+p); starts rounded down to 32-boundaries
    for a in range(0, len(leftover), 2):
        ops = []
        jm = [-1] * 128
        im = [leftover[a][0]] * 128
        il0, js0, ln0 = leftover[a]
        pe = js0 - 384
        p0 = 32 * (pe // 32)
        ops.append((p0, ln0 + pe - p0, "zt3", il0))
        for p in range(p0, pe + ln0):
            jm[p] = 384 + p
            im[p] = il0
        if a + 1 < len(leftover):
            il1, js1, ln1 = leftover[a + 1]
            pe = js1 - 384 + 64
            p0 = 64 + 32 * ((js1 - 384) // 32)
            ops.append((p0, ln1 + pe - p0, "t3b", il1))
            for p in range(p0, pe + ln1):
                jm[p] = 320 + p
                im[p] = il1
        tiles.append(ops); jmaps.append(jm); ilmaps.append(im)

    # split ops so each stays in an aligned partition block:
    # base 0 -> <=128, base 64 -> <=64, base 32/96 -> <=32
    def lim(b):
        return 128 if b == 0 else (64 if b == 64 else 32)
    tiles2 = []
    for ops in tiles:
        o2 = []
        for (p0, plen, src, il) in ops:
            while plen > 0:
                take = min(plen, {0: 128, 32: 32, 64: 64, 96: 32}[p0])
                o2.append((p0, take, src, il))
                p0 += take
                plen -= take
        tiles2.append(o2)
    return tiles2, jmaps, ilmaps


TILES, JMAPS, ILMAPS = _build_recipe()
NCH = len(TILES)


def _perm(i):
    """Position of true z-row i inside ag_out."""
    if i < 384:
        u = i // 16
        return 54 * (u % 8) + 16 * (u // 8) + (i % 16)
    e = i - 384
    return 54 * (e // 6) + 48 + (e % 6)


def _build_module():
    import concourse.bass as bass
    import concourse.mybir as mybir
    import concourse.tile as tile
    from concourse import bacc
    from concourse.masks import make_identity

    dt = mybir.dt
    nc = bacc.Bacc("TRN2", target_bir_lowering=False, debug=False,
                   num_devices=NCORES)

    # ---- per-core DRAM inputs ----
    wsb_d = nc.dram_tensor("wsb", [128, NCH * O], dt.bfloat16,
                           kind="ExternalInput").ap()
    zidx = nc.dram_tensor("zidx", [NIL, 1], dt.int32, kind="ExternalInput").ap()
    zjidx = nc.dram_tensor("zjidx", [128, 4], dt.int32, kind="ExternalInput").ap()
    embs = {}
    for nm in ("es0", "es1", "es2", "es3a", "es3b"):
        embs[nm] = nc.dram_tensor(nm, [CARD, ED], dt.bfloat16,
                                  kind="ExternalInput").ap()
    idxq = nc.dram_tensor("idxq", [128, 20], dt.int32, kind="ExternalInput").ap()
    dfT = nc.dram_tensor("dfT", [16, BATCH], dt.bfloat16, kind="ExternalInput").ap()
    dw0 = nc.dram_tensor("dw0", [16, 512], dt.bfloat16, kind="ExternalInput").ap()
    dw1 = nc.dram_tensor("dw1", [512, 256], dt.bfloat16, kind="ExternalInput").ap()
    dw2 = nc.dram_tensor("dw2", [256, 64], dt.bfloat16, kind="ExternalInput").ap()
    dw3 = nc.dram_tensor("dw3", [64, 16], dt.bfloat16, kind="ExternalInput").ap()
    db0q = nc.dram_tensor("db0q", [128, 4], dt.float32, kind="ExternalInput").ap()
    db1q = nc.dram_tensor("db1q", [128, 2], dt.float32, kind="ExternalInput").ap()
    db2q = nc.dram_tensor("db2q", [64, 1], dt.float32, kind="ExternalInput").ap()
    db3q = nc.dram_tensor("db3q", [16, 1], dt.float32, kind="ExternalInput").ap()
    pw1 = nc.dram_tensor("pw1", [512, 256], dt.bfloat16, kind="ExternalInput").ap()
    pw2 = nc.dram_tensor("pw2", [256, 1], dt.bfloat16, kind="ExternalInput").ap()
    pb0q = nc.dram_tensor("pb0q", [128, 4], dt.float32, kind="ExternalInput").ap()
    pb1q = nc.dram_tensor("pb1q", [128, 2], dt.float32, kind="ExternalInput").ap()
    pb2q = nc.dram_tensor("pb2q", [1, 1], dt.float32, kind="ExternalInput").ap()
    out_d = nc.dram_tensor("out", [1, BATCH], dt.float32, kind="ExternalOutput").ap()

    rg = [list(range(NCORES))]

    with tile.TileContext(nc) as tc:
        with tc.tile_pool(name="const", bufs=1) as cp, \
             tc.tile_pool(name="dram", bufs=1, space="DRAM") as dp:

            ag_in = dp.tile([NIL, BATCH], dt.bfloat16, tag="ag_in", name="ag_in")
            ag_out = dp.tile([ZR, BATCH], dt.bfloat16, tag="ag_out", name="ag_out")
            zi_d = dp.tile([NIL, BATCH], dt.bfloat16, tag="zi_d", name="zi_d")
            ar_in = dp.tile([O, BATCH], dt.bfloat16, tag="ar_in", name="ar_in")
            ar_out = dp.tile([O, BATCH], dt.bfloat16, tag="ar_out", name="ar_out")

            # weight slab SBUF home; prefetch DMAs are issued on the scalar
            # HWDGE ring (below) so they never delay the sync-ring front-end
            wsb = cp.tile([128, NCH * O], dt.bfloat16, tag="wsb", name="wsb")

            # ---- constants / small weights ----
            ident = cp.tile([128, 128], dt.bfloat16, tag="ident", name="ident")
            make_identity(nc, ident[:])
            idx_sb = cp.tile([128, 20], dt.int32, tag="idx_sb", name="idx_sb")
            nc.sync.dma_start(out=idx_sb[:], in_=idxq[:, :])
            zidx_sb = cp.tile([NIL, 1], dt.int32, tag="zidx_sb", name="zidx_sb")
            nc.sync.dma_start(out=zidx_sb[:], in_=zidx[:, :])
            zjidx_sb = cp.tile([128, 4], dt.int32, tag="zjidx_sb", name="zjidx_sb")
            nc.sync.dma_start(out=zjidx_sb[:], in_=zjidx[:, :])
            dfT_sb = cp.tile([16, BATCH], dt.bfloat16, tag="dfT_sb", name="dfT_sb")
            nc.sync.dma_start(out=dfT_sb[:], in_=dfT[:, :])
            dw0_sb = cp.tile([16, 512], dt.bfloat16, tag="dw0_sb", name="dw0_sb")
            nc.sync.dma_start(out=dw0_sb[:], in_=dw0[:, :])
            dw1_sb = [cp.tile([128, 256], dt.bfloat16, tag=f"dw1_{k}", name=f"dw1_{k}")
                      for k in range(4)]
            for k in range(4):
                nc.sync.dma_start(out=dw1_sb[k][:], in_=dw1[k * 128:(k + 1) * 128, :])
            dw2_sb = [cp.tile([128, 64], dt.bfloat16, tag=f"dw2_{k}", name=f"dw2_{k}")
                      for k in range(2)]
            for k in range(2):
                nc.sync.dma_start(out=dw2_sb[k][:], in_=dw2[k * 128:(k + 1) * 128, :])
            dw3_sb = cp.tile([64, 16], dt.bfloat16, tag="dw3_sb", name="dw3_sb")
            nc.sync.dma_start(out=dw3_sb[:], in_=dw3[:, :])
            pw1_sb = [cp.tile([128, 256], dt.bfloat16, tag=f"pw1_{k}", name=f"pw1_{k}")
                      for k in range(4)]
            for k in range(4):
                nc.sync.dma_start(out=pw1_sb[k][:], in_=pw1[k * 128:(k + 1) * 128, :])
            pw2_sb = [cp.tile([128, 1], dt.bfloat16, tag=f"pw2_{k}", name=f"pw2_{k}")
                      for k in range(2)]
            for k in range(2):
                nc.sync.dma_start(out=pw2_sb[k][:], in_=pw2[k * 128:(k + 1) * 128, :])
            db0_sb = cp.tile([128, 4], dt.float32, tag="db0_sb", name="db0_sb")
            nc.sync.dma_start(out=db0_sb[:], in_=db0q[:, :])
            db1_sb = cp.tile([128, 2], dt.float32, tag="db1_sb", name="db1_sb")
            nc.sync.dma_start(out=db1_sb[:], in_=db1q[:, :])
            db2_sb = cp.tile([64, 1], dt.float32, tag="db2_sb", name="db2_sb")
            nc.sync.dma_start(out=db2_sb[:], in_=db2q[:, :])
            db3_sb = cp.tile([16, 1], dt.float32, tag="db3_sb", name="db3_sb")
            nc.sync.dma_start(out=db3_sb[:], in_=db3q[:, :])
            pb0_sb = cp.tile([128, 4], dt.float32, tag="pb0_sb", name="pb0_sb")
            nc.sync.dma_start(out=pb0_sb[:], in_=pb0q[:, :])
            pb1_sb = cp.tile([128, 2], dt.float32, tag="pb1_sb", name="pb1_sb")
            nc.sync.dma_start(out=pb1_sb[:], in_=pb1q[:, :])
            pb2_sb = cp.tile([1, 1], dt.float32, tag="pb2_sb", name="pb2_sb")
            nc.sync.dma_start(out=pb2_sb[:], in_=pb2q[:, :])

            # ---- gathers: slots 0-2 single table, slot 3 = two-half piece ----
            with tc.tile_pool(name="gather", bufs=1) as gp, \
                 tc.tile_pool(name="ps_g", bufs=1, space="PSUM") as pg, \
                 tc.tile_pool(name="ps_d", bufs=2, space="PSUM") as pd, \
                 tc.tile_pool(name="dmlp", bufs=1) as dm:
                zsl = []
                for s in range(4):
                    ps_s = pg.tile([16, BATCH], dt.bfloat16, tag=f"psg{s}",
                                   name=f"psg{s}")
                    for bc in range(4):
                        gt = gp.tile([128, ED], dt.bfloat16, tag=f"g{s}_{bc}",
                                     name=f"g{s}_{bc}")
                        if s < 3:
                            nc.gpsimd.indirect_dma_start(
                                out=gt[:], out_offset=None, in_=embs[f"es{s}"][:, :],
                                in_offset=bass.IndirectOffsetOnAxis(
                                    ap=idx_sb[:, s * 4 + bc:s * 4 + bc + 1], axis=0))
                        else:
                            nc.gpsimd.indirect_dma_start(
                                out=gt[:], out_offset=None, in_=embs["es3a"][:, :],
                                in_offset=bass.IndirectOffsetOnAxis(
                                    ap=idx_sb[:, 12 + bc:13 + bc], axis=0))
                            nc.gpsimd.indirect_dma_start(
                                out=gt[:], out_offset=None, in_=embs["es3b"][:, :],
                                in_offset=bass.IndirectOffsetOnAxis(
                                    ap=idx_sb[:, 16 + bc:17 + bc], axis=0),
                                compute_op=mybir.AluOpType.add)
                        nc.tensor.transpose(out=ps_s[:16, bc * 128:(bc + 1) * 128],
                                            in_=gt[:, :], identity=ident[:])
                    z_s = dm.tile([16, BATCH], dt.bfloat16, tag=f"zsl{s}",
                                  name=f"zsl{s}")
                    nc.vector.tensor_copy(out=z_s[:], in_=ps_s[:16, :])
                    zsl.append(z_s)

                # ---- dense MLP (exactly 0 on cores != 0 via zeroed dw3/db3) ----
                h1 = []
                for mc in range(4):
                    d1 = pd.tile([128, BATCH], dt.float32, tag="dscr", name="d1")
                    nc.tensor.matmul(out=d1[:], lhsT=dw0_sb[:, mc * 128:(mc + 1) * 128],
                                     rhs=dfT_sb[:], start=True, stop=True)
                    h = dm.tile([128, BATCH], dt.bfloat16, tag=f"h1_{mc}",
                                name=f"h1_{mc}")
                    nc.scalar.activation(out=h[:], in_=d1[:],
                                         func=mybir.ActivationFunctionType.Relu,
                                         bias=db0_sb[:, mc:mc + 1])
                    h1.append(h)
                h2 = []
                for mc in range(2):
                    d2 = pd.tile([128, BATCH], dt.float32, tag="dscr", name="d2")
                    for k in range(4):
                        nc.tensor.matmul(out=d2[:],
                                         lhsT=dw1_sb[k][:, mc * 128:(mc + 1) * 128],
                                         rhs=h1[k][:], start=(k == 0), stop=(k == 3))
                    h = dm.tile([128, BATCH], dt.bfloat16, tag=f"h2_{mc}",
                                name=f"h2_{mc}")
                    nc.scalar.activation(out=h[:], in_=d2[:],
                                         func=mybir.ActivationFunctionType.Relu,
                                         bias=db1_sb[:, mc:mc + 1])
                    h2.append(h)
                d3 = pd.tile([128, BATCH], dt.float32, tag="dscr", name="d3")
                for k in range(2):
                    nc.tensor.matmul(out=d3[:64, :], lhsT=dw2_sb[k][:, :],
                                     rhs=h2[k][:], start=(k == 0), stop=(k == 1))
                h3 = dm.tile([64, BATCH], dt.bfloat16, tag="h3", name="h3")
                nc.scalar.activation(out=h3[:], in_=d3[:64, :],
                                     func=mybir.ActivationFunctionType.Relu,
                                     bias=db2_sb[:, 0:1])
                d4 = pd.tile([128, BATCH], dt.float32, tag="dscr", name="d4")
                nc.tensor.matmul(out=d4[:16, :], lhsT=dw3_sb[:, :], rhs=h3[:],
                                 start=True, stop=True)
                dense_sb = dm.tile([16, BATCH], dt.bfloat16, tag="dense_sb",
                                   name="dense_sb")
                nc.vector.tensor_scalar_add(out=dense_sb[:], in0=d4[:16, :],
                                            scalar1=db3_sb[:, 0:1])
                nc.vector.tensor_add(out=zsl[0][:], in0=zsl[0][:], in1=dense_sb[:])

                for s in range(3):
                    nc.sync.dma_start(out=ag_in[16 * s:16 * s + 16, :], in_=zsl[s][:])
                nc.sync.dma_start(out=ag_in[48:54, :], in_=zsl[3][0:6, :])

            # weight slab prefetch on the scalar HWDGE ring (own ring, so
            # it streams during gathers/AG without delaying the sync ring)
            PFC = 8  # chunks per prefetch DMA
            for t0 in range(0, NCH, PFC):
                t1 = min(NCH, t0 + PFC)
                nc.scalar.dma_start(out=wsb[:, t0 * O:t1 * O],
                                    in_=wsb_d[:, t0 * O:t1 * O])

            nc.gpsimd.collective_compute(
                "AllGather", mybir.AluOpType.bypass, replica_groups=rg,
                ins=[ag_in[:].opt()], outs=[ag_out[:].opt()])

            # per-core z_i rows -> DRAM bounce for broadcast reads (first so
            # the b_t pipeline can start), then zT j-tiles in true z-order,
            # all via Q7 indirect gathers (frees the sync ring for b_t DMAs)
            ziT = cp.tile([NIL, BATCH], dt.bfloat16, tag="ziT", name="ziT")
            nc.gpsimd.indirect_dma_start(
                out=ziT[:], out_offset=None, in_=ag_out[:, :],
                in_offset=bass.IndirectOffsetOnAxis(ap=zidx_sb[:, 0:1], axis=0))
            nc.sync.dma_start(out=zi_d[:, :], in_=ziT[:])

            zt = []
            for jc in range(3):
                t = cp.tile([128, BATCH], dt.bfloat16, tag=f"zt{jc}", name=f"zt{jc}")
                nc.gpsimd.indirect_dma_start(
                    out=t[:], out_offset=None, in_=ag_out[:, :],
                    in_offset=bass.IndirectOffsetOnAxis(
                        ap=zjidx_sb[:, jc:jc + 1], axis=0))
                zt.append(t)
            t3full = cp.tile([112, BATCH], dt.bfloat16, tag="t3f", name="t3f")
            nc.gpsimd.indirect_dma_start(
                out=t3full[:], out_offset=None, in_=ag_out[:, :],
                in_offset=bass.IndirectOffsetOnAxis(
                    ap=zjidx_sb[0:112, 3:4], axis=0))
            zt.append(t3full)
            t3b = t3full

            # ---- main loop: exact-triangle chunks ----
            with tc.tile_pool(name="bp", bufs=6) as bp, \
                 tc.tile_pool(name="ip", bufs=4) as ip, \
                 tc.tile_pool(name="ps_acc", bufs=1, space="PSUM") as pa, \
                 tc.tile_pool(name="outp", bufs=1) as op_:

                acc = [pa.tile([128, BATCH], dt.float32, tag=f"acc{oc}",
                               name=f"acc{oc}") for oc in range(4)]

                srcs = {"zt0": zt[0], "zt1": zt[1], "zt2": zt[2],
                        "zt3": zt[3], "t3b": t3b}
                # zero the it ring once so gap partitions are finite
                its = [ip.tile([128, BATCH], dt.bfloat16, tag="it", name="it")
                       for _ in range(4)]
                for t in its:
                    nc.vector.memset(t[:], 0)

                cur_il = -1
                b = None
                segctr = 0
                for k in range(NCH):
                    it = its[k % 4] if k < 4 else ip.tile(
                        [128, BATCH], dt.bfloat16, tag="it", name="it")
                    for (p0, plen, src, il) in TILES[k]:
                        if il != cur_il:
                            b = bp.tile([128, BATCH], dt.bfloat16, tag="b_t",
                                        name="b_t")
                            nc.sync.dma_start(
                                out=b[:],
                                in_=zi_d[il:il + 1, :].to_broadcast([128, BATCH]))
                            cur_il = il
                        eng = nc.gpsimd if segctr % 3 == 2 else nc.vector
                        eng.tensor_mul(out=it[p0:p0 + plen, :],
                                       in0=srcs[src][p0:p0 + plen, :],
                                       in1=b[p0:p0 + plen, :])
                        segctr += 1
                    for oc in range(4):
                        nc.tensor.matmul(
                            out=acc[oc][:],
                            lhsT=wsb[:, k * O + oc * 128:k * O + (oc + 1) * 128],
                            rhs=it[:, :],
                            start=(k == 0), stop=(k == NCH - 1))

                for oc in range(4):
                    osb = op_.tile([128, BATCH], dt.bfloat16, tag=f"osb{oc}",
                                   name=f"osb{oc}")
                    nc.scalar.activation(out=osb[:], in_=acc[oc][:],
                                         func=mybir.ActivationFunctionType.Copy)
                    nc.scalar.dma_start(out=ar_in[oc * 128:(oc + 1) * 128, :],
                                        in_=osb[:])

            nc.gpsimd.collective_compute(
                "AllReduce", mybir.AluOpType.add, replica_groups=rg,
                ins=[ar_in[:].opt()], outs=[ar_out[:].opt()])

            # ---- prediction MLP tail ----
            with tc.tile_pool(name="tail_sb", bufs=1) as ts, \
                 tc.tile_pool(name="ps_t", bufs=1, space="PSUM") as pt:
                h0 = []
                for kc in range(4):
                    r = ts.tile([128, BATCH], dt.bfloat16, tag=f"red{kc}",
                                name=f"red{kc}")
                    nc.sync.dma_start(out=r[:], in_=ar_out[kc * 128:(kc + 1) * 128, :])
                    h = ts.tile([128, BATCH], dt.bfloat16, tag=f"h0_{kc}",
                                name=f"h0_{kc}")
                    nc.scalar.activation(out=h[:], in_=r[:],
                                         func=mybir.ActivationFunctionType.Relu,
                                         bias=pb0_sb[:, kc:kc + 1])
                    h0.append(h)
                h1p = []
                for mc in range(2):
                    p1 = pt.tile([128, BATCH], dt.float32, tag=f"p1_{mc}",
                                 name=f"p1_{mc}")
                    for kc in range(4):
                        nc.tensor.matmul(out=p1[:],
                                         lhsT=pw1_sb[kc][:, mc * 128:(mc + 1) * 128],
                                         rhs=h0[kc][:], start=(kc == 0), stop=(kc == 3))
                    h = ts.tile([128, BATCH], dt.bfloat16, tag=f"h1p_{mc}",
                                name=f"h1p_{mc}")
                    nc.scalar.activation(out=h[:], in_=p1[:],
                                         func=mybir.ActivationFunctionType.Relu,
                                         bias=pb1_sb[:, mc:mc + 1])
                    h1p.append(h)
                p2 = pt.tile([1, BATCH], dt.float32, tag="p2", name="p2")
                for mc in range(2):
                    nc.tensor.matmul(out=p2[:], lhsT=pw2_sb[mc][:, :], rhs=h1p[mc][:],
                                     start=(mc == 0), stop=(mc == 1))
                res = ts.tile([1, BATCH], dt.float32, tag="res", name="res")
                nc.scalar.activation(out=res[:], in_=p2[:],
                                     func=mybir.ActivationFunctionType.Sigmoid,
                                     bias=pb2_sb[:, 0:1])
                nc.sync.dma_start(out=out_d[:, :], in_=res[:])

    nc.compile()
    return nc


def _host_prep(inputs):
    f32 = np.float32
    df = np.asarray(inputs["dense_features"], f32)
    sf = np.asarray(inputs["sparse_features"])
    emb = np.asarray(inputs["emb"], f32)
    pw0 = np.asarray(inputs["pw0"], f32)

    idx = ((sf.astype(np.int64) + 1) % CARD).astype(np.int32)   # [512, 26]
    embb = emb.astype(BF16)                                     # [26, CARD, 16]

    # folded symmetric interaction weights (diag unfolded), bf16
    pw0v = pw0.reshape(ZR, ZR, O)
    Wfull = pw0v + pw0v.transpose(1, 0, 2)
    ar = np.arange(ZR)
    Wfull[ar, ar] = pw0v[ar, ar]
    Wb = Wfull.astype(BF16)                                     # [432, 432, 512]
    del Wfull

    dfT = np.zeros((16, BATCH), BF16)
    dfT[:13] = df.T.astype(BF16)
    dw0p = np.zeros((16, 512), f32)
    dw0p[:13] = np.asarray(inputs["dw0"], f32)

    def col(b, p):
        return np.asarray(b, f32).reshape(p, 128).T.copy()

    zj = np.zeros((128, 4), np.int32)
    for jc in range(3):
        for p in range(128):
            zj[p, jc] = _perm(128 * jc + p)
    for p in range(48):
        zj[p, 3] = _perm(384 + p)
    for p in range(64, 112):
        zj[p, 3] = _perm(384 + p - 64)

    common = {
        "zjidx": zj,
        "dfT": dfT,
        "dw0": dw0p.astype(BF16),
        "dw1": np.asarray(inputs["dw1"], f32).astype(BF16),
        "dw2": np.asarray(inputs["dw2"], f32).astype(BF16),
        "db0q": col(inputs["db0"], 4),
        "db1q": col(inputs["db1"], 2),
        "db2q": np.asarray(inputs["db2"], f32).reshape(64, 1).copy(),
        "pw1": np.asarray(inputs["pw1"], f32).astype(BF16),
        "pw2": np.asarray(inputs["pw2"], f32).reshape(256, 1).astype(BF16),
        "pb0q": col(inputs["pb0"], 4),
        "pb1q": col(inputs["pb1"], 2),
        "pb2q": np.asarray(inputs["pb2"], f32).reshape(1, 1).copy(),
    }
    dw3 = np.asarray(inputs["dw3"], f32).astype(BF16)
    db3 = np.asarray(inputs["db3"], f32).reshape(16, 1).astype(f32)
    zero_tab = np.zeros((CARD, ED), BF16)
    zero_idx = np.zeros(BATCH, np.int32)

    # c-independent row template from the tile maps
    Jt = np.asarray(JMAPS, np.int64).reshape(-1)       # [NCH*128], -1 = gap
    ILt = np.asarray(ILMAPS, np.int64).reshape(-1)
    JSt = np.array([_jstart(il) for il in range(NIL)], np.int64)[ILt]

    in_maps = []
    for c in range(NCORES):
        m = dict(common)
        m["dw3"] = dw3 if c == 0 else np.zeros_like(dw3)
        m["db3q"] = db3 if c == 0 else np.zeros_like(db3)

        I = JSt + c                          # i = jstart + c
        Wc = Wb[I, np.maximum(Jt, 0)]        # [NCH*128, 512] bf16
        Wc[(Jt < 0) | (Jt < I)] = 0
        m["wsb"] = np.ascontiguousarray(
            Wc.reshape(NCH, RPC, O).transpose(1, 0, 2).reshape(128, NCH * O))

        m["zidx"] = np.array([[_perm(_i_of(il, c))] for il in range(NIL)],
                             np.int32)

        idx_cols = []
        for s in range(3):
            u = [c, 8 + c, 16 + c][s]   # unit; u==0 is dense
            if u == 0:
                m[f"es{s}"] = zero_tab
                idx_cols.append(zero_idx)
            else:
                m[f"es{s}"] = np.ascontiguousarray(embb[u - 1])
                idx_cols.append(idx[:, u - 1])
        # piece: cols e=0..5 <- table 23+(6c+e)//16, dim (6c+e)%16
        ta = 23 + (6 * c) // 16
        ea = np.zeros((CARD, ED), BF16)
        eb = np.zeros((CARD, ED), BF16)
        tb = None
        for e in range(6):
            t_ = 23 + (6 * c + e) // 16
            d_ = (6 * c + e) % 16
            if t_ == ta:
                ea[:, e] = embb[t_][:, d_]
            else:
                tb = t_
                eb[:, e] = embb[t_][:, d_]
        m["es3a"] = ea
        m["es3b"] = eb
        idx_cols.append(idx[:, ta])
        idx_cols.append(idx[:, tb] if tb is not None else zero_idx)

        iq = np.zeros((128, 20), np.int32)
        for sa in range(5):
            iq[:, sa * 4:(sa + 1) * 4] = idx_cols[sa].reshape(4, 128).T
        m["idxq"] = iq
        in_maps.append(m)
    return in_maps


def kernel(**inputs):
    from concourse import bass_utils
    import os

    if "nc" not in _state:
        _state["nc"] = _build_module()
    in_maps = _host_prep(inputs)
    trace = bool(int(os.environ.get("DLRM_TRACE", "0")))
    res = bass_utils.run_bass_kernel_spmd(
        _state["nc"], in_maps, core_ids=list(range(NCORES)), trace=trace)
    _state["last_results"] = res
    return np.asarray(res.results[0]["out"], np.float32).reshape(BATCH)
